# revision 67
# baseline (speedup 1.0000x reference)
"""Causal self-attention (B=4, T=2048, C=1024, H=16, D=64) on 8 TRN2 NeuronCores.

Sharding: core i handles batch b = i//2 and head-group g = i%2 (8 of the 16
heads).  Each core computes the QKV projection for its batch restricted to its
heads' columns, runs causal attention for its 8 heads, and produces a partial
output projection y_part = ctx_g @ w_out[rows of g].  The two partials per
batch are summed on the host (y[b] = y_part[2b] + y_part[2b+1]).

Single software-pipelined phase: the tensor engine's in-order queue is fed so
it never idles (idle gaps drop the PE to the mid p-state, 2x slower):
  - QKV projection (bf16) is split into (n, mc) units; the units for the
    first half of the sequence run up front, the rest are drained as filler
    between attention steps of tq-block 0.
  - Attention per (head, jb-block): QK^T (scores transposed, [tk, tq]) ->
    exp on the scalar engine -> PV with stationary [v|ones] (denominator
    rides along rows 64..127) -> reciprocal+mul normalize on DVE.
  - PV for head h-1 is emitted right after QK of head h, so the exp latency
    of head h-1 is hidden behind head h's score matmuls and filler.
  - The output projection of jb-block 0 drains as filler inside jb-block 1.
exp() skips max-subtraction (scores here are |s| < ~10; raw exp is safe).
"""

from contextlib import ExitStack

import numpy as np
import ml_dtypes

import concourse.bass as bass
import concourse.mybir as mybir
from concourse import bacc, tile
from concourse.bass_utils import run_bass_kernel_spmd

F32 = mybir.dt.float32
BF16 = mybir.dt.bfloat16

B, T, C = 4, 2048, 1024
H, D = 16, 64
N_CORES = 8


def build_core_program(R=T, HPC=8, C_=C):
    KC = C_ // 128            # contraction chunks for QKV matmul
    SUBS = HPC // 2           # 128-row groups per q/k/v section of qkv_T
    MC = 3 * SUBS             # 128-col chunks of this core's w_qkv slice
    CTXC = HPC * D            # ctx channels owned by this core
    OKC = CTXC // 128         # contraction chunks for out-proj
    NCH = R // 128            # tk/tq 128-chunks
    TQ = min(512, R)          # qkv matmul moving width
    NT = R // TQ
    BLK = min(1024, R)        # tq block width for attention/out-proj
    NB = R // BLK
    PW = min(512, BLK)        # PSUM piece width
    GPB = BLK // 128          # 128-row output groups per block
    EXP = mybir.ActivationFunctionType.Exp

    nc = bacc.Bacc("TRN2", target_bir_lowering=False, debug=False)

    # inputs arrive pre-tiled for single-issue DMAs: [partition, chunk, free]
    x_t = nc.dram_tensor("x_t", [128, KC, R], BF16, kind="ExternalInput")
    w_qkv_c = nc.dram_tensor("w_qkv_c", [128, KC, 3 * CTXC], BF16,
                             kind="ExternalInput")
    w_out_c = nc.dram_tensor("w_out_c", [128, OKC, C_], BF16,
                             kind="ExternalInput")
    y_part = nc.dram_tensor("y_part", [R, C_], BF16, kind="ExternalOutput")

    with tile.TileContext(nc) as tc:
        with (
            tc.tile_pool(name="const", bufs=1) as constp,
            tc.tile_pool(name="qkv", bufs=1) as qkvp,
            tc.tile_pool(name="vall", bufs=1) as vallp,
            tc.tile_pool(name="ctxT", bufs=1) as ctxTp,
            tc.tile_pool(name="wout", bufs=1) as woutp,
            tc.tile_pool(name="attn", bufs=2) as attnp,
            tc.tile_pool(name="recsb", bufs=2) as recp,
        ):
            # attention PSUM pools, closed before the out-proj tail so its
            # pool can buffer deeply
            attn_psum = ExitStack()
            sps = attn_psum.enter_context(
                tc.tile_pool(name="scoresps", bufs=2, space="PSUM"))
            cpsp = attn_psum.enter_context(
                tc.tile_pool(name="ctxps", bufs=2, space="PSUM"))
            qT = qkvp.tile([128, SUBS, R], BF16)
            kT = qkvp.tile([128, SUBS, R], BF16)
            # v_all[:, sub, i, half, :] = [v_half (64) | ones (64)]: the PV
            # stationary for head 2*sub+half, tk chunk i.  The ones columns
            # replicate the softmax denominator across PSUM partitions
            # 64..127 so normalization is full-width on DVE.
            v_all = vallp.tile([128, SUBS, NCH, 2, 128], BF16)
            ctx_T = ctxTp.tile([128, OKC, R], BF16)
            w_out_sb = woutp.tile([128, OKC, C_], BF16)

            with (
                tc.tile_pool(name="wp", bufs=1) as wp,
                tc.tile_pool(name="xp", bufs=2) as xp,
                tc.tile_pool(name="qkvps", bufs=2, space="PSUM") as qps,
            ):
                w_sb = wp.tile([128, KC, 3 * CTXC], BF16)
                x_tiles = {}

                def ensure_x(n):
                    if n in x_tiles or n >= NT:
                        return
                    # per-kc DMAs spread the transfer over many queues; the
                    # sync engine is otherwise idle
                    x_sb = xp.tile([128, KC, TQ], BF16, name="x_sb", tag="x")
                    for kc in range(KC):
                        nc.sync.dma_start(
                            out=x_sb[:, kc, :],
                            in_=x_t[:, kc, n * TQ:(n + 1) * TQ])
                    x_tiles[n] = x_sb

                def qkv_unit(n, mc):
                    def emit():
                        ensure_x(n)
                        ensure_x(n + 1)
                        ps = qps.tile([128, TQ], F32, name="qkv_ps",
                                      tag="qkv_ps")
                        for kc in range(KC):
                            nc.tensor.matmul(
                                ps,
                                lhsT=w_sb[:, kc, 128 * mc:128 * (mc + 1)],
                                rhs=x_tiles[n][:, kc, :],
                                start=(kc == 0), stop=(kc == KC - 1),
                            )
                        sec, sub = mc // SUBS, mc % SUBS
                        dest = (qT, kT)[sec]
                        nc.vector.tensor_copy(
                            out=dest[:, sub, n * TQ:(n + 1) * TQ], in_=ps
                        )
                    return emit

                def vdir_unit(n, js):
                    # v computed directly in [t, ch] orientation (lhsT = x):
                    # exactly the layout PV wants, no transposes needed
                    def emit():
                        i = n * (TQ // 128) + js
                        ps = qps.tile([128, TQ], F32, name="qkv_ps",
                                      tag="qkv_ps")
                        for kc in range(KC):
                            nc.tensor.matmul(
                                ps[:, :CTXC],
                                lhsT=x_tiles[n][:, kc,
                                                128 * js:128 * (js + 1)],
                                rhs=w_sb[:, kc, 2 * CTXC:3 * CTXC],
                                start=(kc == 0), stop=(kc == KC - 1),
                            )
                        # column order (head-major) matches (sub, half, d)
                        nc.vector.tensor_copy(
                            out=v_all[:, :, i, :, 0:64], in_=ps[:, :CTXC])
                    return emit

                # ---- schedule ----
                filler = []

                def drain(k):
                    for _ in range(min(k, len(filler))):
                        filler.pop(0)()

                # startup input DMAs: the first unit's w columns and x go
                # out first in small pieces, spread over all three
                # DMA-capable engines, so the first matmuls start early and
                # later column groups stream in just ahead of their units
                x0 = xp.tile([128, KC, TQ], BF16, name="x_sb", tag="x")
                x_tiles[0] = x0
                issues = []
                for kc in range(KC):
                    issues.append((w_sb[:, kc, 0:128], w_qkv_c[:, kc, 0:128]))
                    issues.append((x0[:, kc, :], x_t[:, kc, 0:TQ]))
                # group boundaries follow consumption order: q columns,
                # then k (units 2*SUBS..), then v (vdir rhs) — each group
                # transfers while the previous group's units run
                bounds = sorted({b for b in (128, 512, 2 * CTXC)
                                 if b < 3 * CTXC} | {3 * CTXC})
                for a, b in zip(bounds, bounds[1:]):
                    for kc in range(KC):
                        issues.append((w_sb[:, kc, a:b], w_qkv_c[:, kc, a:b]))
                engs = [nc.sync, nc.gpsimd, nc.scalar]
                for j, (o, i_) in enumerate(issues):
                    engs[j % 3].dma_start(out=o, in_=i_)
                for sub in range(SUBS):  # DVE is idle this early
                    nc.vector.memset(v_all[:, sub, :, :, 64:128], 1.0)

                # upfront QKV work: everything needed for jb block 0
                n_up = max(1, (BLK + TQ - 1) // TQ)  # n chunks for jb0
                for n in range(NT):
                    units = [qkv_unit(n, mc) for mc in range(2 * SUBS)]
                    units += [vdir_unit(n, js) for js in range(TQ // 128)]
                    if n < n_up:
                        for u in units:
                            u()
                    else:
                        filler.extend(units)

                pad_done = set()  # (jb, i, slot) pads already zeroed

                def attn_step(h, jb):
                    """Emit QK+exp for (h, jb); return a PV closure."""
                    blo, bhi = BLK * jb, BLK * (jb + 1)
                    sub, p0 = h // 2, 64 * (h % 2)
                    half = h % 2
                    qh = qT[p0:p0 + 64, sub, :]
                    kh = kT[p0:p0 + 64, sub, :]
                    chunks = [i for i in range(NCH) if 128 * i < bhi]
                    # interleave filler between chunk QK/exp pairs so the
                    # tensor engine has work while exp catches up; jb 0
                    # spreads the whole queue evenly, later blocks hold
                    # filler back for the end-of-block exp tail
                    if not filler:
                        per_head = 0
                    elif jb == 0:
                        per_head = (len(filler) + HPC - 1 - h) // (HPC - h)
                    elif h == 0:
                        # the previous block's last PV/normalize is emitted
                        # inside this step; out-proj filler reading that
                        # ctx_T range must not be drained before it
                        per_head = 0
                    else:
                        # steady-state heads get two filler units (they run
                        # slightly scalar-bound with only one); the last two
                        # heads rely on the explicit back-load drains
                        per_head = 2 if h < HPC - 2 else 0
                    di = max(2, len(chunks) // per_head) if per_head else 0
                    # emission units: "wide" chunks (c0 < PW, left-padded with
                    # zeros to column 0 so they stream every PV piece in
                    # full) go alone; "narrow" chunks (c0 >= PW) are merged
                    # into shared sc/attn tiles so one exp call covers a run
                    units = []  # (members, tile_width); member = (i, off, c0, width)
                    cur, cw = [], 0
                    # chunks starting mid-piece need zero-padding only when
                    # piece 0 would otherwise lack two full-coverage chunks
                    # to open and close its accumulation group
                    nz = sum(1 for i in chunks if 128 * i <= blo)
                    for i in chunks:
                        c0 = max(blo, 128 * i) - blo
                        width = BLK - c0
                        if 0 < c0 < PW and nz < 2:
                            units.append(([(i, c0, c0, width)], BLK))
                        elif c0 == 0:
                            units.append(([(i, 0, 0, width)], BLK))
                        else:
                            if cur and cw + width > BLK:
                                units.append((cur, cw))
                                cur, cw = [], 0
                            cur.append((i, cw, c0, width))
                            cw += width
                    if cur:
                        units.append((cur, cw))
                    at_info = {}  # i -> (tile, off, c0, is_wide)
                    for uc, (members, twidth) in enumerate(units):
                        at = attnp.tile([128, twidth], BF16, name=f"at{uc}",
                                        tag=f"attn{uc}")
                        sc = sps.tile([128, BLK], F32, name="sc_ps",
                                      tag="sc_ps")
                        pad = members[0][1]  # wide: pad == c0; narrow: 0
                        if pad and (jb, members[0][0], h % 2) not in pad_done:
                            # ring slot pads stay zero across reuses: exp only
                            # ever writes [pad:] for this (jb, chunk)
                            pad_done.add((jb, members[0][0], h % 2))
                            nc.gpsimd.memset(at[:, 0:pad], 0.0)
                        for (i, off, c0, width) in members:
                            p = off  # split at psum bank boundaries
                            while p < off + width:
                                e = min(off + width, (p // 512 + 1) * 512)
                                nc.tensor.matmul(
                                    sc[:, p:e],
                                    lhsT=kh[:, 128 * i:128 * (i + 1)],
                                    rhs=qh[:, blo + c0 + p - off:
                                            blo + c0 + e - off],
                                    start=True, stop=True,
                                )
                                p = e
                        lo_col = members[0][1]
                        hi_col = members[-1][1] + members[-1][3]
                        nc.scalar.activation(at[:, lo_col:hi_col],
                                             sc[:, lo_col:hi_col],
                                             EXP, scale=0.125)
                        for (i, off, c0, width) in members:
                            if 128 * i >= blo:  # diag block: zero upper-tri
                                nc.gpsimd.affine_select(
                                    out=at[:, off:off + 128],
                                    in_=at[:, off:off + 128],
                                    compare_op=mybir.AluOpType.is_ge,
                                    fill=0.0, base=0,
                                    pattern=[[1, 128]], channel_multiplier=-1,
                                )
                            # cov0: tile covers the block from column 0
                            # (single chunk at off==c0, zero-padded below)
                            cov0 = len(members) == 1 and off == c0
                            at_info[i] = (at, off, c0, cov0)
                        if di and uc % di == di - 1:
                            drain(1)

                    def pv():
                        cps_tiles = {}
                        for p in range(0, BLK, PW):
                            cps = cpsp.tile([128, PW], F32, name="ctx_ps",
                                            tag="ctx_ps")
                            cps_tiles[p] = cps
                            # full-coverage chunks stream the whole piece
                            # (wide ones via their zero pad); partial chunks
                            # accumulate a sub-range mid-group, so a full
                            # chunk must open and another must close it
                            full = [i for i in chunks
                                    if at_info[i][3] or at_info[i][2] <= p]
                            part = [i for i in chunks
                                    if p < at_info[i][2] < p + PW
                                    and not at_info[i][3]]
                            order = [full[0]] + part + full[1:]
                            for idx, i in enumerate(order):
                                at, off, c0, is_wide = at_info[i]
                                orig = c0 - off  # tq of tile column 0
                                s = p if i in full else c0
                                nc.tensor.matmul(
                                    cps[:, s - p:PW],
                                    lhsT=v_all[:, sub, i, half, :],
                                    rhs=at[:, s - orig:p + PW - orig],
                                    start=(idx == 0),
                                    stop=(idx == len(order) - 1),
                                    skip_group_check=(s != p),
                                )
                            # normalize: ctx/denom into ctx_T (bf16)
                            rec = recp.tile([128, PW], F32, name="rec",
                                            tag="rec")
                            nc.vector.reciprocal_approx_fast(out=rec, in_=cps)
                            nc.vector.tensor_mul(
                                ctx_T[p0:p0 + 64, sub, blo + p:blo + p + PW],
                                cps[0:64, :],
                                rec[64:128, :],
                            )
                    return pv

                # QK of head h is emitted before PV of head h-1: PV(h-1)
                # waits on the tail of exp(h-1), while QK(h) can start at
                # once — by the time QK(h) drains, exp(h-1) is long done
                prev_pv = None
                for h in range(HPC):
                    pv_new = attn_step(h, 0)
                    if prev_pv is not None:
                        prev_pv()
                    prev_pv = pv_new
                drain(len(filler))  # finish QKV before jb1; covers last exp

            def outproj_unit(gm, nn, ypool, epool, y_eng):
                def emit():
                    yp = ypool.tile([128, 512], F32, name="y_ps", tag="y_ps")
                    for kc in range(OKC):
                        nc.tensor.matmul(
                            yp,
                            lhsT=ctx_T[:, kc, 128 * gm:128 * (gm + 1)],
                            rhs=w_out_sb[:, kc, 512 * nn:512 * (nn + 1)],
                            start=(kc == 0), stop=(kc == OKC - 1),
                        )
                    ye = epool.tile([128, 512], BF16, name="ye", tag="ye")
                    nc.vector.tensor_copy(out=ye, in_=yp)
                    for q in range(2):  # spread issue cost over idle engines
                        y_eng[(2 * (gm * (C_ // 512) + nn) + q)
                              % len(y_eng)].dma_start(
                            out=y_part[128 * gm:128 * (gm + 1),
                                       512 * nn + 256 * q:
                                       512 * nn + 256 * (q + 1)],
                            in_=ye[:, 256 * q:256 * (q + 1)],
                        )
                return emit

            nc.sync.dma_start(out=w_out_sb, in_=w_out_c[:, :, :])
            with (
                tc.tile_pool(name="yev", bufs=2) as yevp,
                tc.tile_pool(name="yps", bufs=2, space="PSUM") as yps,
            ):
                for jb in range(1, NB):
                    # scalar stays clear of mid-run y issues: it is the
                    # near-critical engine during attention blocks
                    filler.extend(outproj_unit(GPB * (jb - 1) + m, nn,
                                               yps, yevp,
                                               [nc.sync, nc.gpsimd])
                                  for m in range(GPB)
                                  for nn in range(C_ // 512))
                    for h in range(HPC):
                        pv_new = attn_step(h, jb)
                        if prev_pv is not None:
                            prev_pv()
                        prev_pv = pv_new
                        if h >= HPC - 2:  # back-load: cover the last exps
                            drain(3)
                drain(len(filler))  # leftovers also cover the final exp
                if prev_pv is not None:
                    prev_pv()

            # attention pools released: the final block's out-proj gets a
            # deep PSUM ring so its matmul groups stream without stalls
            attn_psum.close()
            with (
                tc.tile_pool(name="yev2", bufs=4) as yevp2,
                tc.tile_pool(name="yps2", bufs=4, space="PSUM") as yps2,
            ):
                for m in range(GPB):
                    for nn in range(C_ // 512):
                        outproj_unit(GPB * (NB - 1) + m, nn, yps2, yevp2,
                                     [nc.sync, nc.gpsimd, nc.scalar])()

    nc.finalize()
    return nc


def _tile_rows(a):
    """[C, F] -> [128, C//128, F] with row c at [c % 128, c // 128]."""
    c, f = a.shape
    return np.ascontiguousarray(a.reshape(c // 128, 128, f).transpose(1, 0, 2))


def make_in_maps(x, w_qkv, w_out):
    x = np.asarray(x, dtype=np.float32)
    w_qkv = np.asarray(w_qkv, dtype=np.float32)
    w_out = np.asarray(w_out, dtype=np.float32)
    in_maps = []
    for core in range(N_CORES):
        b, g = core // 2, core % 2
        cols = slice(512 * g, 512 * (g + 1))
        wq = np.concatenate(
            [w_qkv[:, cols], w_qkv[:, 1024:][:, cols], w_qkv[:, 2048:][:, cols]],
            axis=1,
        )
        in_maps.append({
            "x_t": _tile_rows(x[b].T).astype(ml_dtypes.bfloat16),
            "w_qkv_c": _tile_rows(wq).astype(ml_dtypes.bfloat16),
            "w_out_c": _tile_rows(
                w_out[512 * g:512 * (g + 1), :]).astype(ml_dtypes.bfloat16),
        })
    return in_maps


_NC_CACHE = None
LAST_RESULT = None


def kernel(x, w_qkv, w_out):
    global _NC_CACHE, LAST_RESULT
    if _NC_CACHE is None:
        _NC_CACHE = build_core_program()
    nc = _NC_CACHE
    in_maps = make_in_maps(x, w_qkv, w_out)
    res = run_bass_kernel_spmd(nc, in_maps, list(range(N_CORES)))
    LAST_RESULT = res
    outs = [np.asarray(r["y_part"], dtype=np.float32) for r in res.results]
    y = np.stack([outs[2 * b] + outs[2 * b + 1] for b in range(B)], axis=0)
    return y.astype(np.float32)
